# revision 9
# baseline (speedup 1.0000x reference)
"""Multi-head attention on 8 trn2 NeuronCores, head-parallel (2 heads/core).

Math per head h (reference semantics):
  Q = query @ Wq[h] + bq[h];  K = key @ Wk[h] + bk[h];  V = query @ Wv[h] + bv[h]
  P = exp(Q K^T / sqrt(D));  alpha = P / rowsum(P)
  ctx = alpha @ V;  y_h = (ctx @ Wp[h] + bp[h]) @ Wo[h]
  out = sum_h y_h + bo

Device-side formulation (all layouts transposed, f32 storage, f32r matmuls):
  Per core: project QT/KT/VT = W^T @ xT per head, attention with unnormalized
  softmax (rowsum via ones-matmul, normalization folded after PV), output
  y^T[e, tok] partial = sum_{h in core} W_h^T @ ctxn_h + bias ** ones, where
  W_h = Wp[h] @ Wo[h] (host-premultiplied) and bias collects bv/bp/bo terms.
  ReduceScatter across the 8 cores; host concatenates the shards.
"""

import sys

if "/opt/trn_rl_repo" not in sys.path:
    sys.path.insert(0, "/opt/trn_rl_repo")

import numpy as np

import concourse.mybir as mybir
import concourse.tile as tile
from concourse import bacc
from concourse.bass_utils import run_bass_kernel_spmd

B, S = 4, 2048
IN, D, H = 1024, 128, 16
NCORES = 8
HPC = H // NCORES  # heads per core
NCH = IN // 128  # input chunks
TB = 512  # projection token block
NTB = S // TB
QB = 512  # attention query block
NQB = S // QB
KT = 128  # attention key tile
NKT = S // KT
ESH = D // NCORES  # output shard rows per core

f32 = mybir.dt.float32
f32r = mybir.dt.float32r
AF = mybir.ActivationFunctionType

_cache = {}


def build():
    nc = bacc.Bacc(None, target_bir_lowering=False, num_devices=NCORES)

    qT = nc.dram_tensor("qT", [B, IN, S], f32r, kind="ExternalInput")
    kT = nc.dram_tensor("kT", [B, IN, S], f32r, kind="ExternalInput")
    wq = nc.dram_tensor("wq", [HPC, IN, D], f32r, kind="ExternalInput")
    wk = nc.dram_tensor("wk", [HPC, IN, D], f32r, kind="ExternalInput")
    wv = nc.dram_tensor("wv", [HPC, IN, D], f32r, kind="ExternalInput")
    wh = nc.dram_tensor("wh", [HPC, D, D], f32r, kind="ExternalInput")
    bqT = nc.dram_tensor("bqT", [D, HPC], f32, kind="ExternalInput")
    bkT = nc.dram_tensor("bkT", [D, HPC], f32, kind="ExternalInput")
    biasv = nc.dram_tensor("biasv", [1, D], f32r, kind="ExternalInput")
    oner = nc.dram_tensor("oner", [1, QB], f32r, kind="ExternalInput")
    onem = nc.dram_tensor("onem", [D, D], f32r, kind="ExternalInput")

    out_y = nc.dram_tensor("out_y", [ESH, B * S], f32, kind="ExternalOutput")
    y_bounce = [nc.dram_tensor(f"y_bounce{b}", [D, S], f32) for b in range(B)]
    y_shard = [nc.dram_tensor(f"y_shard{b}", [ESH, S], f32) for b in range(B)]

    scale = 1.0 / float(np.sqrt(D))

    with tile.TileContext(nc) as tc:
        with (
            tc.tile_pool(name="const", bufs=1) as cpool,
            tc.tile_pool(name="xch", bufs=12) as xch,
            tc.tile_pool(name="qkv", bufs=2) as qkv,
            tc.tile_pool(name="work", bufs=2) as work,
            tc.tile_pool(name="pexpp", bufs=4) as pexpp,
            tc.tile_pool(name="ps", bufs=2, space="PSUM") as ps,
        ):
            # ---- resident constants ----
            wq_sb = cpool.tile([128, HPC, NCH, D], f32r, tag="wq_sb")
            wk_sb = cpool.tile([128, HPC, NCH, D], f32r, tag="wk_sb")
            wv_sb = cpool.tile([128, HPC, NCH, D], f32r, tag="wv_sb")
            for sb_t, dram_t in ((wq_sb, wq), (wk_sb, wk), (wv_sb, wv)):
                for h in range(HPC):
                    for c in range(NCH):
                        nc.sync.dma_start(
                            sb_t[:, h, c, :], dram_t[h, c * 128 : (c + 1) * 128, :]
                        )
            wh_sb = cpool.tile([128, HPC, D], f32r, tag="wh_sb")
            for h in range(HPC):
                nc.sync.dma_start(wh_sb[:, h, :], wh[h])
            bq_sb = cpool.tile([128, HPC], f32, tag="bq_sb")
            bk_sb = cpool.tile([128, HPC], f32, tag="bk_sb")
            nc.sync.dma_start(bq_sb[:], bqT[:])
            nc.sync.dma_start(bk_sb[:], bkT[:])
            biasv_sb = cpool.tile([1, D], f32r, tag="biasv_sb")
            oner_sb = cpool.tile([1, QB], f32r, tag="oner_sb")
            onem_sb = cpool.tile([D, D], f32r, tag="onem_sb")
            nc.sync.dma_start(biasv_sb[:], biasv[:])
            nc.sync.dma_start(oner_sb[:], oner[:])
            nc.sync.dma_start(onem_sb[:], onem[:])
            ident = cpool.tile([128, 128], f32, tag="ident")
            from concourse.masks import make_identity

            make_identity(nc, ident[:])

            for b in range(B):
                # ---- projections: Q & V from qT, K from kT ----
                QT = [qkv.tile([128, S], f32r, tag=f"QT{h}", name=f"QT{h}") for h in range(HPC)]
                KTs = [qkv.tile([128, S], f32r, tag=f"KT{h}", name=f"KT{h}") for h in range(HPC)]
                Vn = [qkv.tile([128, S], f32r, tag=f"VN{h}", name=f"VN{h}") for h in range(HPC)]

                for tb in range(NTB):
                    sl = slice(tb * TB, (tb + 1) * TB)
                    chs = []
                    for c in range(NCH):
                        ch = xch.tile([128, TB], f32r, tag="xch")
                        nc.sync.dma_start(ch[:], qT[b, c * 128 : (c + 1) * 128, sl])
                        chs.append(ch)
                    for h in range(HPC):
                        pq = ps.tile([128, TB], f32, tag="pC", name="pq")
                        for c in range(NCH):
                            nc.tensor.matmul(
                                pq[:], wq_sb[:, h, c, :], chs[c][:],
                                start=(c == 0), stop=(c == NCH - 1),
                            )
                        with nc.allow_low_precision(reason="f32r PE operand"):
                            nc.vector.tensor_scalar_add(
                                QT[h][:, sl], pq[:], bq_sb[:, h : h + 1]
                            )
                    for h in range(HPC):
                        pv = ps.tile([128, TB], f32, tag="pZ", name="pv", bufs=2)
                        for c in range(NCH):
                            nc.tensor.matmul(
                                pv[:], wv_sb[:, h, c, :], chs[c][:],
                                start=(c == 0), stop=(c == NCH - 1),
                            )
                        vstage = work.tile([128, TB], f32, tag=f"vstage{h}")
                        nc.vector.tensor_copy(vstage[:], pv[:])
                        for j in range(TB // 128):
                            ptr = ps.tile([128, 128], f32, tag="pZ", name="ptr", bufs=2)
                            nc.tensor.transpose(
                                ptr[:], vstage[:, j * 128 : (j + 1) * 128], ident[:]
                            )
                            col = tb * TB + j * 128
                            with nc.allow_low_precision(reason="f32r PE operand"):
                                nc.vector.tensor_copy(Vn[h][:, col : col + 128], ptr[:])

                for tb in range(NTB):
                    sl = slice(tb * TB, (tb + 1) * TB)
                    chs = []
                    for c in range(NCH):
                        ch = xch.tile([128, TB], f32r, tag="xch")
                        nc.sync.dma_start(ch[:], kT[b, c * 128 : (c + 1) * 128, sl])
                        chs.append(ch)
                    for h in range(HPC):
                        pk = ps.tile([128, TB], f32, tag="pC", name="pk")
                        for c in range(NCH):
                            nc.tensor.matmul(
                                pk[:], wk_sb[:, h, c, :], chs[c][:],
                                start=(c == 0), stop=(c == NCH - 1),
                            )
                        with nc.allow_low_precision(reason="f32r PE operand"):
                            nc.vector.tensor_scalar_add(
                                KTs[h][:, sl], pk[:], bk_sb[:, h : h + 1]
                            )

                # ---- attention: qblock pairs share 2-bank psum + one wide exp ----
                GPS_KT = set(range(11, NKT))  # acc adds on gpsimd for these kt
                for qbp in range(NQB // 2):
                    q0 = qbp * 2 * QB
                    sl0 = slice(q0, q0 + QB)
                    sl1 = slice(q0 + QB, q0 + 2 * QB)
                    slp = slice(q0, q0 + 2 * QB)
                    pzs = [
                        ps.tile([128, QB], f32, tag="pZ", name="pz", bufs=2)
                        for _ in range(2)
                    ]
                    for h in range(HPC):
                        pctx0 = ps.tile([128, QB], f32, tag="pC", name="pctx0")
                        pctx1 = ps.tile([128, QB], f32, tag="pC", name="pctx1")
                        acc_d = work.tile([128, 2 * QB], f32r, tag="acc_d", name="acc_d")
                        acc_g = work.tile([128, 2 * QB], f32r, tag="acc_g", name="acc_g")
                        first_d, first_g = True, True
                        for kt in range(NKT):
                            ps2 = ps.tile([128, 2 * QB], f32, tag="pS", name="ps2", bufs=2)
                            ksl = slice(kt * 128, (kt + 1) * 128)
                            nc.tensor.matmul(
                                ps2[:, :QB], KTs[h][:, ksl], QT[h][:, sl0],
                                start=True, stop=True,
                            )
                            nc.tensor.matmul(
                                ps2[:, QB:], KTs[h][:, ksl], QT[h][:, sl1],
                                start=True, stop=True,
                            )
                            pexp = pexpp.tile([128, 2 * QB], f32r, tag="pexp")
                            nc.scalar.activation(pexp[:], ps2[:], AF.Exp, scale=scale)
                            nc.tensor.matmul(
                                pctx0[:], Vn[h][:, ksl], pexp[:, :QB],
                                start=(kt == 0), stop=(kt == NKT - 1),
                            )
                            nc.tensor.matmul(
                                pctx1[:], Vn[h][:, ksl], pexp[:, QB:],
                                start=(kt == 0), stop=(kt == NKT - 1),
                            )
                            with nc.allow_low_precision(reason="f32r PE operand"):
                                if kt in GPS_KT:
                                    if first_g:
                                        nc.gpsimd.tensor_copy(acc_g[:], pexp[:])
                                        first_g = False
                                    else:
                                        nc.gpsimd.tensor_add(acc_g[:], acc_g[:], pexp[:])
                                else:
                                    if first_d:
                                        nc.vector.tensor_copy(acc_d[:], pexp[:])
                                        first_d = False
                                    else:
                                        nc.vector.tensor_add(acc_d[:], acc_d[:], pexp[:])
                        # rowsum+broadcast via all-ones matmul, both halves
                        pbc = ps.tile([128, 2 * QB], f32, tag="pS", name="pbc", bufs=2)
                        for half, hsl in ((0, slice(0, QB)), (1, slice(QB, 2 * QB))):
                            nc.tensor.matmul(
                                pbc[:, hsl], onem_sb[:], acc_d[:, hsl],
                                start=True, stop=False,
                            )
                            nc.tensor.matmul(
                                pbc[:, hsl], onem_sb[:], acc_g[:, hsl],
                                start=False, stop=True,
                            )
                        rsbr = work.tile([128, 2 * QB], f32, tag="rsbr", name="rsbr", bufs=1)
                        nc.vector.reciprocal_approx_fast(out=rsbr[:], in_=pbc[:])
                        ctxn = work.tile([128, 2 * QB], f32r, tag="ctxn", name="ctxn")
                        with nc.allow_low_precision(reason="f32r PE operand"):
                            nc.vector.tensor_mul(ctxn[:, :QB], pctx0[:], rsbr[:, :QB])
                            nc.vector.tensor_mul(ctxn[:, QB:], pctx1[:], rsbr[:, QB:])
                        nc.tensor.matmul(
                            pzs[0][:], wh_sb[:, h, :], ctxn[:, :QB],
                            start=(h == 0), stop=False,
                        )
                        nc.tensor.matmul(
                            pzs[1][:], wh_sb[:, h, :], ctxn[:, QB:],
                            start=(h == 0), stop=False,
                        )
                    for half in range(2):
                        nc.tensor.matmul(
                            pzs[half][:], biasv_sb[:], oner_sb[:], start=False, stop=True
                        )
                        ytile = work.tile([128, QB], f32, tag="ytile")
                        nc.vector.tensor_copy(ytile[:], pzs[half][:])
                        col = q0 + half * QB
                        nc.sync.dma_start(y_bounce[b][:, col : col + QB], ytile[:])

                nc.gpsimd.collective_compute(
                    "ReduceScatter",
                    mybir.AluOpType.add,
                    replica_groups=[list(range(NCORES))],
                    ins=[y_bounce[b][:].opt()],
                    outs=[y_shard[b][:].opt()],
                )
                nc.sync.dma_start(out_y[:, b * S : (b + 1) * S], y_shard[b][:])


    nc.compile()
    return nc


def kernel(**inputs):
    query = np.asarray(inputs["query"], np.float32)
    key = np.asarray(inputs["key"], np.float32)
    Wq, bq = np.asarray(inputs["Wq"], np.float32), np.asarray(inputs["bq"], np.float32)
    Wk, bk = np.asarray(inputs["Wk"], np.float32), np.asarray(inputs["bk"], np.float32)
    Wv, bv = np.asarray(inputs["Wv"], np.float32), np.asarray(inputs["bv"], np.float32)
    Wp, bp = np.asarray(inputs["Wp"], np.float32), np.asarray(inputs["bp"], np.float32)
    Wo, bo = np.asarray(inputs["Wo"], np.float32), np.asarray(inputs["bo"], np.float32)

    qT = np.ascontiguousarray(query.transpose(0, 2, 1))  # [B, IN, S]
    kT = np.ascontiguousarray(key.transpose(0, 2, 1))

    if "nc" not in _cache:
        _cache["nc"] = build()
    nc = _cache["nc"]

    in_maps = []
    for i in range(NCORES):
        hs = slice(i * HPC, (i + 1) * HPC)
        Wo_h = Wo.reshape(H, D, D)  # rows of Wo per head
        wh = np.einsum(
            "hde,hef->hdf",
            Wp[hs].astype(np.float64),
            Wo_h[hs].astype(np.float64),
        ).astype(np.float32)
        bias = (
            np.einsum("hd,hdf->f", bv[hs].astype(np.float64), wh.astype(np.float64))
            + np.einsum(
                "hd,hdf->f", bp[hs].astype(np.float64), Wo_h[hs].astype(np.float64)
            )
            + bo.astype(np.float64) / NCORES
        ).astype(np.float32)
        in_maps.append(
            {
                "qT": qT,
                "kT": kT,
                "wq": np.ascontiguousarray(Wq[hs]),
                "wk": np.ascontiguousarray(Wk[hs]),
                "wv": np.ascontiguousarray(Wv[hs]),
                "wh": wh,
                "bqT": np.ascontiguousarray(bq[hs].T),
                "bkT": np.ascontiguousarray(bk[hs].T),
                "biasv": bias.reshape(1, D),
                "oner": np.ones((1, QB), np.float32),
                "onem": np.ones((D, D), np.float32),
            }
        )

    res = run_bass_kernel_spmd(nc, in_maps, core_ids=list(range(NCORES)))
    _cache["last_result"] = res
    yT = np.concatenate([res.results[i]["out_y"] for i in range(NCORES)], axis=0)
    return np.ascontiguousarray(yT.T).reshape(B, S, D)


# revision 11
# speedup vs baseline: 1.0833x; 1.0833x over previous
"""Multi-head attention on 8 trn2 NeuronCores, head-parallel (2 heads/core).

Math per head h (reference semantics):
  Q = query @ Wq[h] + bq[h];  K = key @ Wk[h] + bk[h];  V = query @ Wv[h] + bv[h]
  P = exp(Q K^T / sqrt(D));  alpha = P / rowsum(P)
  ctx = alpha @ V;  y_h = (ctx @ Wp[h] + bp[h]) @ Wo[h]
  out = sum_h y_h + bo

Device-side formulation (all layouts transposed, f32 storage, f32r matmuls):
  Per core: project QT/KT/VT = W^T @ xT per head, attention with unnormalized
  softmax (rowsum via ones-matmul, normalization folded after PV), output
  y^T[e, tok] partial = sum_{h in core} W_h^T @ ctxn_h + bias ** ones, where
  W_h = Wp[h] @ Wo[h] (host-premultiplied) and bias collects bv/bp/bo terms.
  ReduceScatter across the 8 cores; host concatenates the shards.
"""

import sys

if "/opt/trn_rl_repo" not in sys.path:
    sys.path.insert(0, "/opt/trn_rl_repo")

import numpy as np

import concourse.mybir as mybir
import concourse.tile as tile
from concourse import bacc
from concourse.bass_utils import run_bass_kernel_spmd

B, S = 4, 2048
IN, D, H = 1024, 128, 16
NCORES = 8
HPC = H // NCORES  # heads per core
NCH = IN // 128  # input chunks
TB = 512  # projection token block
NTB = S // TB
QB = 512  # attention query block
NQB = S // QB
KT = 128  # attention key tile
NKT = S // KT
ESH = D // NCORES  # output shard rows per core

f32 = mybir.dt.float32
f32r = mybir.dt.float32r
AF = mybir.ActivationFunctionType

_cache = {}


def build():
    nc = bacc.Bacc(None, target_bir_lowering=False, num_devices=NCORES)

    qT = nc.dram_tensor("qT", [B, IN, S], f32r, kind="ExternalInput")
    kT = nc.dram_tensor("kT", [B, IN, S], f32r, kind="ExternalInput")
    wq = nc.dram_tensor("wq", [HPC, IN, D], f32r, kind="ExternalInput")
    wk = nc.dram_tensor("wk", [HPC, IN, D], f32r, kind="ExternalInput")
    wv = nc.dram_tensor("wv", [HPC, IN, D], f32r, kind="ExternalInput")
    wh = nc.dram_tensor("wh", [HPC, D, D], f32r, kind="ExternalInput")
    bqT = nc.dram_tensor("bqT", [D, HPC], f32, kind="ExternalInput")
    bkT = nc.dram_tensor("bkT", [D, HPC], f32, kind="ExternalInput")
    biasv = nc.dram_tensor("biasv", [1, D], f32r, kind="ExternalInput")
    oner = nc.dram_tensor("oner", [1, QB], f32r, kind="ExternalInput")
    onem = nc.dram_tensor("onem", [D, D], f32r, kind="ExternalInput")

    out_y = nc.dram_tensor("out_y", [ESH, B * S], f32, kind="ExternalOutput")
    y_bounce = [nc.dram_tensor(f"y_bounce{b}", [D, S], f32) for b in range(B)]
    y_shard = [nc.dram_tensor(f"y_shard{b}", [ESH, S], f32) for b in range(B)]

    scale = 1.0 / float(np.sqrt(D))

    with tile.TileContext(nc) as tc:
        with (
            tc.tile_pool(name="const", bufs=1) as cpool,
            tc.tile_pool(name="xch", bufs=12) as xch,
            tc.tile_pool(name="qkv", bufs=2) as qkv,
            tc.tile_pool(name="work", bufs=2) as work,
            tc.tile_pool(name="pexpp", bufs=4) as pexpp,
            tc.tile_pool(name="ps", bufs=2, space="PSUM") as ps,
        ):
            # ---- resident constants ----
            wq_sb = cpool.tile([128, HPC, NCH, D], f32r, tag="wq_sb")
            wk_sb = cpool.tile([128, HPC, NCH, D], f32r, tag="wk_sb")
            wv_sb = cpool.tile([128, HPC, NCH, D], f32r, tag="wv_sb")
            for sb_t, dram_t in ((wq_sb, wq), (wk_sb, wk), (wv_sb, wv)):
                for h in range(HPC):
                    for c in range(NCH):
                        nc.sync.dma_start(
                            sb_t[:, h, c, :], dram_t[h, c * 128 : (c + 1) * 128, :]
                        )
            wh_sb = cpool.tile([128, HPC, D], f32r, tag="wh_sb")
            for h in range(HPC):
                nc.sync.dma_start(wh_sb[:, h, :], wh[h])
            bq_sb = cpool.tile([128, HPC], f32, tag="bq_sb")
            bk_sb = cpool.tile([128, HPC], f32, tag="bk_sb")
            nc.sync.dma_start(bq_sb[:], bqT[:])
            nc.sync.dma_start(bk_sb[:], bkT[:])
            biasv_sb = cpool.tile([1, D], f32r, tag="biasv_sb")
            oner_sb = cpool.tile([1, QB], f32r, tag="oner_sb")
            onem_sb = cpool.tile([D, D], f32r, tag="onem_sb")
            nc.sync.dma_start(biasv_sb[:], biasv[:])
            nc.sync.dma_start(oner_sb[:], oner[:])
            nc.sync.dma_start(onem_sb[:], onem[:])
            ident = cpool.tile([128, 128], f32, tag="ident")
            from concourse.masks import make_identity

            make_identity(nc, ident[:])

            for b in range(B):
                # ---- projections: Q & V from qT, K from kT ----
                QT = [qkv.tile([128, S], f32r, tag=f"QT{h}", name=f"QT{h}") for h in range(HPC)]
                KTs = [qkv.tile([128, S], f32r, tag=f"KT{h}", name=f"KT{h}") for h in range(HPC)]
                Vn = [qkv.tile([128, S], f32r, tag=f"VN{h}", name=f"VN{h}") for h in range(HPC)]

                for tb in range(NTB):
                    sl = slice(tb * TB, (tb + 1) * TB)
                    chs = []
                    for c in range(NCH):
                        ch = xch.tile([128, TB], f32r, tag="xch")
                        nc.sync.dma_start(ch[:], qT[b, c * 128 : (c + 1) * 128, sl])
                        chs.append(ch)
                    pq = ps.tile([128, 2 * TB], f32, tag="pS", name="pq", bufs=2)
                    for h in range(HPC):
                        for c in range(NCH):
                            nc.tensor.matmul(
                                pq[:, h * TB : (h + 1) * TB],
                                wq_sb[:, h, c, :], chs[c][:],
                                start=(c == 0), stop=(c == NCH - 1),
                            )
                    for h in range(HPC):
                        with nc.allow_low_precision(reason="f32r PE operand"):
                            nc.vector.tensor_scalar_add(
                                QT[h][:, sl], pq[:, h * TB : (h + 1) * TB],
                                bq_sb[:, h : h + 1],
                            )
                    pv = ps.tile([128, 2 * TB], f32, tag="pS", name="pv", bufs=2)
                    for h in range(HPC):
                        for c in range(NCH):
                            nc.tensor.matmul(
                                pv[:, h * TB : (h + 1) * TB],
                                wv_sb[:, h, c, :], chs[c][:],
                                start=(c == 0), stop=(c == NCH - 1),
                            )
                    for h in range(HPC):
                        vstage = work.tile([128, TB], f32, tag=f"vstage{h}")
                        nc.vector.tensor_copy(vstage[:], pv[:, h * TB : (h + 1) * TB])
                        for j in range(TB // 128):
                            ptr = ps.tile([128, 128], f32, tag="pZ", name="ptr", bufs=2)
                            nc.tensor.transpose(
                                ptr[:], vstage[:, j * 128 : (j + 1) * 128], ident[:]
                            )
                            col = tb * TB + j * 128
                            with nc.allow_low_precision(reason="f32r PE operand"):
                                nc.vector.tensor_copy(Vn[h][:, col : col + 128], ptr[:])

                for tb in range(NTB):
                    sl = slice(tb * TB, (tb + 1) * TB)
                    chs = []
                    for c in range(NCH):
                        ch = xch.tile([128, TB], f32r, tag="xch")
                        nc.sync.dma_start(ch[:], kT[b, c * 128 : (c + 1) * 128, sl])
                        chs.append(ch)
                    pk = ps.tile([128, 2 * TB], f32, tag="pS", name="pk", bufs=2)
                    for h in range(HPC):
                        for c in range(NCH):
                            nc.tensor.matmul(
                                pk[:, h * TB : (h + 1) * TB],
                                wk_sb[:, h, c, :], chs[c][:],
                                start=(c == 0), stop=(c == NCH - 1),
                            )
                    for h in range(HPC):
                        with nc.allow_low_precision(reason="f32r PE operand"):
                            nc.vector.tensor_scalar_add(
                                KTs[h][:, sl], pk[:, h * TB : (h + 1) * TB],
                                bk_sb[:, h : h + 1],
                            )

                # ---- attention: qblock pairs share 2-bank psum + one wide exp ----
                GPS_KT = set(range(0, 4))  # gpsimd takes early kt (tail-free)
                for qbp in range(NQB // 2):
                    q0 = qbp * 2 * QB
                    sl0 = slice(q0, q0 + QB)
                    sl1 = slice(q0 + QB, q0 + 2 * QB)
                    slp = slice(q0, q0 + 2 * QB)
                    pzs = [
                        ps.tile([128, QB], f32, tag="pZ", name="pz", bufs=2)
                        for _ in range(2)
                    ]
                    for h in range(HPC):
                        pctx0 = ps.tile([128, QB], f32, tag="pC", name="pctx0")
                        pctx1 = ps.tile([128, QB], f32, tag="pC", name="pctx1")
                        acc_d = work.tile([128, 2 * QB], f32r, tag="acc_d", name="acc_d")
                        acc_g = work.tile([128, 2 * QB], f32r, tag="acc_g", name="acc_g")
                        first_d, first_g = True, True
                        prev_d, prev_g = None, None
                        for kt in range(NKT):
                            ps2 = ps.tile([128, 2 * QB], f32, tag="pS", name="ps2", bufs=2)
                            ksl = slice(kt * 128, (kt + 1) * 128)
                            nc.tensor.matmul(
                                ps2[:, :QB], KTs[h][:, ksl], QT[h][:, sl0],
                                start=True, stop=True,
                            )
                            nc.tensor.matmul(
                                ps2[:, QB:], KTs[h][:, ksl], QT[h][:, sl1],
                                start=True, stop=True,
                            )
                            pexp = pexpp.tile([128, 2 * QB], f32r, tag="pexp")
                            nc.scalar.activation(pexp[:], ps2[:], AF.Exp, scale=scale)
                            nc.tensor.matmul(
                                pctx0[:], Vn[h][:, ksl], pexp[:, :QB],
                                start=(kt == 0), stop=(kt == NKT - 1),
                            )
                            nc.tensor.matmul(
                                pctx1[:], Vn[h][:, ksl], pexp[:, QB:],
                                start=(kt == 0), stop=(kt == NKT - 1),
                            )
                            with nc.allow_low_precision(reason="f32r PE operand"):
                                if kt in GPS_KT:
                                    if first_g and prev_g is None:
                                        prev_g = pexp
                                    elif first_g:
                                        nc.gpsimd.tensor_add(
                                            acc_g[:], prev_g[:], pexp[:]
                                        )
                                        first_g = False
                                    else:
                                        nc.gpsimd.tensor_add(
                                            acc_g[:], acc_g[:], pexp[:]
                                        )
                                else:
                                    if first_d and prev_d is None:
                                        prev_d = pexp
                                    elif first_d:
                                        nc.vector.tensor_add(
                                            acc_d[:], prev_d[:], pexp[:]
                                        )
                                        first_d = False
                                    else:
                                        nc.vector.tensor_add(
                                            acc_d[:], acc_d[:], pexp[:]
                                        )
                        # rowsum+broadcast via all-ones matmul, both halves
                        pbc = ps.tile([128, 2 * QB], f32, tag="pS", name="pbc", bufs=2)
                        for half, hsl in ((0, slice(0, QB)), (1, slice(QB, 2 * QB))):
                            nc.tensor.matmul(
                                pbc[:, hsl], onem_sb[:], acc_d[:, hsl],
                                start=True, stop=False,
                            )
                            nc.tensor.matmul(
                                pbc[:, hsl], onem_sb[:], acc_g[:, hsl],
                                start=False, stop=True,
                            )
                        rsbr = work.tile([128, 2 * QB], f32, tag="rsbr", name="rsbr", bufs=1)
                        nc.vector.reciprocal_approx_fast(out=rsbr[:], in_=pbc[:])
                        ctxn = work.tile([128, 2 * QB], f32r, tag="ctxn", name="ctxn")
                        with nc.allow_low_precision(reason="f32r PE operand"):
                            nc.vector.tensor_mul(ctxn[:, :QB], pctx0[:], rsbr[:, :QB])
                            nc.vector.tensor_mul(ctxn[:, QB:], pctx1[:], rsbr[:, QB:])
                        nc.tensor.matmul(
                            pzs[0][:], wh_sb[:, h, :], ctxn[:, :QB],
                            start=(h == 0), stop=False,
                        )
                        nc.tensor.matmul(
                            pzs[1][:], wh_sb[:, h, :], ctxn[:, QB:],
                            start=(h == 0), stop=False,
                        )
                    for half in range(2):
                        nc.tensor.matmul(
                            pzs[half][:], biasv_sb[:], oner_sb[:], start=False, stop=True
                        )
                        ytile = work.tile([128, QB], f32, tag="ytile")
                        nc.vector.tensor_copy(ytile[:], pzs[half][:])
                        col = q0 + half * QB
                        nc.sync.dma_start(y_bounce[b][:, col : col + QB], ytile[:])

                nc.gpsimd.collective_compute(
                    "ReduceScatter",
                    mybir.AluOpType.add,
                    replica_groups=[list(range(NCORES))],
                    ins=[y_bounce[b][:].opt()],
                    outs=[y_shard[b][:].opt()],
                )
                nc.sync.dma_start(out_y[:, b * S : (b + 1) * S], y_shard[b][:])


    nc.compile()
    return nc


def kernel(**inputs):
    query = np.asarray(inputs["query"], np.float32)
    key = np.asarray(inputs["key"], np.float32)
    Wq, bq = np.asarray(inputs["Wq"], np.float32), np.asarray(inputs["bq"], np.float32)
    Wk, bk = np.asarray(inputs["Wk"], np.float32), np.asarray(inputs["bk"], np.float32)
    Wv, bv = np.asarray(inputs["Wv"], np.float32), np.asarray(inputs["bv"], np.float32)
    Wp, bp = np.asarray(inputs["Wp"], np.float32), np.asarray(inputs["bp"], np.float32)
    Wo, bo = np.asarray(inputs["Wo"], np.float32), np.asarray(inputs["bo"], np.float32)

    qT = np.ascontiguousarray(query.transpose(0, 2, 1))  # [B, IN, S]
    kT = np.ascontiguousarray(key.transpose(0, 2, 1))

    if "nc" not in _cache:
        _cache["nc"] = build()
    nc = _cache["nc"]

    in_maps = []
    for i in range(NCORES):
        hs = slice(i * HPC, (i + 1) * HPC)
        Wo_h = Wo.reshape(H, D, D)  # rows of Wo per head
        wh = np.einsum(
            "hde,hef->hdf",
            Wp[hs].astype(np.float64),
            Wo_h[hs].astype(np.float64),
        ).astype(np.float32)
        bias = (
            np.einsum("hd,hdf->f", bv[hs].astype(np.float64), wh.astype(np.float64))
            + np.einsum(
                "hd,hdf->f", bp[hs].astype(np.float64), Wo_h[hs].astype(np.float64)
            )
            + bo.astype(np.float64) / NCORES
        ).astype(np.float32)
        in_maps.append(
            {
                "qT": qT,
                "kT": kT,
                "wq": np.ascontiguousarray(Wq[hs]),
                "wk": np.ascontiguousarray(Wk[hs]),
                "wv": np.ascontiguousarray(Wv[hs]),
                "wh": wh,
                "bqT": np.ascontiguousarray(bq[hs].T),
                "bkT": np.ascontiguousarray(bk[hs].T),
                "biasv": bias.reshape(1, D),
                "oner": np.ones((1, QB), np.float32),
                "onem": np.ones((D, D), np.float32),
            }
        )

    res = run_bass_kernel_spmd(nc, in_maps, core_ids=list(range(NCORES)))
    _cache["last_result"] = res
    yT = np.concatenate([res.results[i]["out_y"] for i in range(NCORES)], axis=0)
    return np.ascontiguousarray(yT.T).reshape(B, S, D)


# revision 14
# speedup vs baseline: 1.1835x; 1.0925x over previous
"""Multi-head attention on 8 trn2 NeuronCores, head-parallel (2 heads/core).

Math per head h (reference semantics):
  Q = query @ Wq[h] + bq[h];  K = key @ Wk[h] + bk[h];  V = query @ Wv[h] + bv[h]
  P = exp(Q K^T / sqrt(D));  alpha = P / rowsum(P)
  ctx = alpha @ V;  y_h = (ctx @ Wp[h] + bp[h]) @ Wo[h]
  out = sum_h y_h + bo

Device-side formulation (all layouts transposed, f32 storage, f32r matmuls):
  Per core: project QT/KT/VT = W^T @ xT per head, attention with unnormalized
  softmax (rowsum via ones-matmul, normalization folded after PV), output
  y^T[e, tok] partial = sum_{h in core} W_h^T @ ctxn_h + bias ** ones, where
  W_h = Wp[h] @ Wo[h] (host-premultiplied) and bias collects bv/bp/bo terms.
  ReduceScatter across the 8 cores; host concatenates the shards.
"""

import sys

if "/opt/trn_rl_repo" not in sys.path:
    sys.path.insert(0, "/opt/trn_rl_repo")

import ml_dtypes
import numpy as np

import concourse.mybir as mybir
import concourse.tile as tile
from concourse import bacc
from concourse.bass_utils import run_bass_kernel_spmd

B, S = 4, 2048
IN, D, H = 1024, 128, 16
NCORES = 8
HPC = H // NCORES  # heads per core
NCH = IN // 128  # input chunks
TB = 512  # projection token block
NTB = S // TB
QB = 512  # attention query block
NQB = S // QB
KT = 128  # attention key tile
NKT = S // KT
ESH = D // NCORES  # output shard rows per core

f32 = mybir.dt.float32
f32r = mybir.dt.float32r
bf16 = mybir.dt.bfloat16
AF = mybir.ActivationFunctionType

_cache = {}


def build():
    nc = bacc.Bacc(None, target_bir_lowering=False, num_devices=NCORES)

    qT = nc.dram_tensor("qT", [B, IN, S], f32r, kind="ExternalInput")
    kT = nc.dram_tensor("kT", [B, IN, S], f32r, kind="ExternalInput")
    wq = nc.dram_tensor("wq", [HPC, IN, D], f32r, kind="ExternalInput")
    wk = nc.dram_tensor("wk", [HPC, IN, D], f32r, kind="ExternalInput")
    wv = nc.dram_tensor("wv", [HPC, IN, D], f32r, kind="ExternalInput")
    wh = nc.dram_tensor("wh", [HPC, D, D], f32r, kind="ExternalInput")
    bqT = nc.dram_tensor("bqT", [D, HPC], f32, kind="ExternalInput")
    bkT = nc.dram_tensor("bkT", [D, HPC], f32, kind="ExternalInput")
    biasv = nc.dram_tensor("biasv", [1, D], f32r, kind="ExternalInput")
    oner = nc.dram_tensor("oner", [1, QB], f32r, kind="ExternalInput")
    onem = nc.dram_tensor("onem", [D, D], f32r, kind="ExternalInput")
    onemb = nc.dram_tensor("onemb", [D, D], bf16, kind="ExternalInput")

    out_y = nc.dram_tensor("out_y", [ESH, B * S], f32, kind="ExternalOutput")
    y_bounce = [nc.dram_tensor(f"y_bounce{b}", [D, S], f32) for b in range(B)]
    y_shard = [nc.dram_tensor(f"y_shard{b}", [ESH, S], f32) for b in range(B)]

    scale = 1.0 / float(np.sqrt(D))

    with tile.TileContext(nc) as tc:
        with (
            tc.tile_pool(name="const", bufs=1) as cpool,
            tc.tile_pool(name="xch", bufs=12) as xch,
            tc.tile_pool(name="qkv", bufs=2) as qkv,
            tc.tile_pool(name="work", bufs=2) as work,
            tc.tile_pool(name="pexpp", bufs=4) as pexpp,
            tc.tile_pool(name="ps", bufs=2, space="PSUM") as ps,
        ):
            # ---- resident constants ----
            wq_sb = cpool.tile([128, HPC, NCH, D], f32r, tag="wq_sb")
            wk_sb = cpool.tile([128, HPC, NCH, D], f32r, tag="wk_sb")
            wv_sb = cpool.tile([128, HPC, NCH, D], f32r, tag="wv_sb")
            for sb_t, dram_t in ((wq_sb, wq), (wk_sb, wk), (wv_sb, wv)):
                for h in range(HPC):
                    for c in range(NCH):
                        nc.sync.dma_start(
                            sb_t[:, h, c, :], dram_t[h, c * 128 : (c + 1) * 128, :]
                        )
            wh_sb = cpool.tile([128, HPC, D], f32r, tag="wh_sb")
            for h in range(HPC):
                nc.sync.dma_start(wh_sb[:, h, :], wh[h])
            bq_sb = cpool.tile([128, HPC], f32, tag="bq_sb")
            bk_sb = cpool.tile([128, HPC], f32, tag="bk_sb")
            nc.sync.dma_start(bq_sb[:], bqT[:])
            nc.sync.dma_start(bk_sb[:], bkT[:])
            biasv_sb = cpool.tile([1, D], f32r, tag="biasv_sb")
            oner_sb = cpool.tile([1, QB], f32r, tag="oner_sb")
            onem_sb = cpool.tile([D, D], f32r, tag="onem_sb")
            onemb_sb = cpool.tile([D, D], bf16, tag="onemb_sb")
            nc.sync.dma_start(onemb_sb[:], onemb[:])
            nc.sync.dma_start(biasv_sb[:], biasv[:])
            nc.sync.dma_start(oner_sb[:], oner[:])
            nc.sync.dma_start(onem_sb[:], onem[:])
            identb = cpool.tile([128, 128], bf16, tag="identb")
            from concourse.masks import make_identity

            make_identity(nc, identb[:])

            for b in range(B):
                # ---- projections: Q & V from qT, K from kT ----
                QT = [qkv.tile([128, S], f32r, tag=f"QT{h}", name=f"QT{h}") for h in range(HPC)]
                KTs = [qkv.tile([128, S], f32r, tag=f"KT{h}", name=f"KT{h}") for h in range(HPC)]
                Vn = [qkv.tile([128, S], bf16, tag=f"VN{h}", name=f"VN{h}") for h in range(HPC)]

                for tb in range(NTB):
                    sl = slice(tb * TB, (tb + 1) * TB)
                    chs = []
                    for c in range(NCH):
                        ch = xch.tile([128, TB], f32r, tag="xch")
                        nc.sync.dma_start(ch[:], qT[b, c * 128 : (c + 1) * 128, sl])
                        chs.append(ch)
                    pq = ps.tile([128, 2 * TB], f32, tag="pS", name="pq", bufs=2)
                    for h in range(HPC):
                        for c in range(NCH):
                            nc.tensor.matmul(
                                pq[:, h * TB : (h + 1) * TB],
                                wq_sb[:, h, c, :], chs[c][:],
                                start=(c == 0), stop=(c == NCH - 1),
                            )
                    for h in range(HPC):
                        with nc.allow_low_precision(reason="f32r PE operand"):
                            nc.vector.tensor_scalar_add(
                                QT[h][:, sl], pq[:, h * TB : (h + 1) * TB],
                                bq_sb[:, h : h + 1],
                            )
                    pv = ps.tile([128, 2 * TB], f32, tag="pS", name="pv", bufs=2)
                    for h in range(HPC):
                        for c in range(NCH):
                            nc.tensor.matmul(
                                pv[:, h * TB : (h + 1) * TB],
                                wv_sb[:, h, c, :], chs[c][:],
                                start=(c == 0), stop=(c == NCH - 1),
                            )
                    for h in range(HPC):
                        vstage = work.tile([128, TB], bf16, tag=f"vstage{h}")
                        with nc.allow_low_precision(reason="bf16 PV operand"):
                            nc.vector.tensor_copy(vstage[:], pv[:, h * TB : (h + 1) * TB])
                        for j in range(TB // 128):
                            ptr = ps.tile([128, 128], bf16, tag="pZ", name="ptr", bufs=2)
                            nc.tensor.transpose(
                                ptr[:], vstage[:, j * 128 : (j + 1) * 128], identb[:]
                            )
                            col = tb * TB + j * 128
                            with nc.allow_low_precision(reason="bf16 PV operand"):
                                nc.vector.tensor_copy(Vn[h][:, col : col + 128], ptr[:])

                for tb in range(NTB):
                    sl = slice(tb * TB, (tb + 1) * TB)
                    chs = []
                    for c in range(NCH):
                        ch = xch.tile([128, TB], f32r, tag="xch")
                        nc.sync.dma_start(ch[:], kT[b, c * 128 : (c + 1) * 128, sl])
                        chs.append(ch)
                    pk = ps.tile([128, 2 * TB], f32, tag="pS", name="pk", bufs=2)
                    for h in range(HPC):
                        for c in range(NCH):
                            nc.tensor.matmul(
                                pk[:, h * TB : (h + 1) * TB],
                                wk_sb[:, h, c, :], chs[c][:],
                                start=(c == 0), stop=(c == NCH - 1),
                            )
                    for h in range(HPC):
                        with nc.allow_low_precision(reason="f32r PE operand"):
                            nc.vector.tensor_scalar_add(
                                KTs[h][:, sl], pk[:, h * TB : (h + 1) * TB],
                                bk_sb[:, h : h + 1],
                            )

                # ---- attention: qblock pairs share 2-bank psum + one wide exp ----
                GPS_KT = set(range(0, 4))  # gpsimd takes early kt (tail-free)
                for qbp in range(NQB // 2):
                    q0 = qbp * 2 * QB
                    sl0 = slice(q0, q0 + QB)
                    sl1 = slice(q0 + QB, q0 + 2 * QB)
                    slp = slice(q0, q0 + 2 * QB)
                    pzs = [
                        ps.tile([128, QB], f32, tag="pZ", name="pz", bufs=2)
                        for _ in range(2)
                    ]
                    for h in range(HPC):
                        pctx0 = ps.tile([128, QB], f32, tag="pC", name="pctx0")
                        pctx1 = ps.tile([128, QB], f32, tag="pC", name="pctx1")
                        acc_d = work.tile([128, 2 * QB], bf16, tag="acc_d", name="acc_d")
                        acc_g = work.tile([128, 2 * QB], bf16, tag="acc_g", name="acc_g")
                        first_d, first_g = True, True
                        prev_d, prev_g = None, None
                        for kt in range(NKT):
                            ps2 = ps.tile([128, 2 * QB], f32, tag="pS", name="ps2", bufs=2)
                            ksl = slice(kt * 128, (kt + 1) * 128)
                            nc.tensor.matmul(
                                ps2[:, :QB], KTs[h][:, ksl], QT[h][:, sl0],
                                start=True, stop=True,
                            )
                            nc.tensor.matmul(
                                ps2[:, QB:], KTs[h][:, ksl], QT[h][:, sl1],
                                start=True, stop=True,
                            )
                            pexp = pexpp.tile([128, 2 * QB], bf16, tag="pexp", bufs=6)
                            nc.scalar.activation(pexp[:], ps2[:], AF.Exp, scale=scale)
                            nc.tensor.matmul(
                                pctx0[:], Vn[h][:, ksl], pexp[:, :QB],
                                start=(kt == 0), stop=(kt == NKT - 1),
                            )
                            nc.tensor.matmul(
                                pctx1[:], Vn[h][:, ksl], pexp[:, QB:],
                                start=(kt == 0), stop=(kt == NKT - 1),
                            )
                            with nc.allow_low_precision(reason="f32r PE operand"):
                                if kt in GPS_KT:
                                    if first_g and prev_g is None:
                                        prev_g = pexp
                                    elif first_g:
                                        nc.gpsimd.tensor_add(
                                            acc_g[:], prev_g[:], pexp[:]
                                        )
                                        first_g = False
                                    else:
                                        nc.gpsimd.tensor_add(
                                            acc_g[:], acc_g[:], pexp[:]
                                        )
                                else:
                                    if first_d and prev_d is None:
                                        prev_d = pexp
                                    elif first_d:
                                        nc.vector.tensor_add(
                                            acc_d[:], prev_d[:], pexp[:]
                                        )
                                        first_d = False
                                    else:
                                        nc.vector.tensor_add(
                                            acc_d[:], acc_d[:], pexp[:]
                                        )
                        # rowsum+broadcast via all-ones matmul, both halves
                        pbc = ps.tile([128, 2 * QB], f32, tag="pS", name="pbc", bufs=2)
                        for half, hsl in ((0, slice(0, QB)), (1, slice(QB, 2 * QB))):
                            nc.tensor.matmul(
                                pbc[:, hsl], onemb_sb[:], acc_d[:, hsl],
                                start=True, stop=False,
                            )
                            nc.tensor.matmul(
                                pbc[:, hsl], onemb_sb[:], acc_g[:, hsl],
                                start=False, stop=True,
                            )
                        rsbr = work.tile([128, 2 * QB], f32, tag="rsbr", name="rsbr", bufs=1)
                        nc.vector.reciprocal_approx_fast(out=rsbr[:], in_=pbc[:])
                        ctxn = work.tile([128, 2 * QB], f32r, tag="ctxn", name="ctxn")
                        with nc.allow_low_precision(reason="f32r PE operand"):
                            nc.vector.tensor_mul(ctxn[:, :QB], pctx0[:], rsbr[:, :QB])
                            nc.vector.tensor_mul(ctxn[:, QB:], pctx1[:], rsbr[:, QB:])
                        nc.tensor.matmul(
                            pzs[0][:], wh_sb[:, h, :], ctxn[:, :QB],
                            start=(h == 0), stop=False,
                        )
                        nc.tensor.matmul(
                            pzs[1][:], wh_sb[:, h, :], ctxn[:, QB:],
                            start=(h == 0), stop=False,
                        )
                    for half in range(2):
                        nc.tensor.matmul(
                            pzs[half][:], biasv_sb[:], oner_sb[:], start=False, stop=True
                        )
                        ytile = work.tile([128, QB], f32, tag="ytile")
                        nc.vector.tensor_copy(ytile[:], pzs[half][:])
                        col = q0 + half * QB
                        nc.sync.dma_start(y_bounce[b][:, col : col + QB], ytile[:])

                nc.gpsimd.collective_compute(
                    "ReduceScatter",
                    mybir.AluOpType.add,
                    replica_groups=[list(range(NCORES))],
                    ins=[y_bounce[b][:].opt()],
                    outs=[y_shard[b][:].opt()],
                )
                nc.sync.dma_start(out_y[:, b * S : (b + 1) * S], y_shard[b][:])


    nc.compile()
    return nc


def kernel(**inputs):
    query = np.asarray(inputs["query"], np.float32)
    key = np.asarray(inputs["key"], np.float32)
    Wq, bq = np.asarray(inputs["Wq"], np.float32), np.asarray(inputs["bq"], np.float32)
    Wk, bk = np.asarray(inputs["Wk"], np.float32), np.asarray(inputs["bk"], np.float32)
    Wv, bv = np.asarray(inputs["Wv"], np.float32), np.asarray(inputs["bv"], np.float32)
    Wp, bp = np.asarray(inputs["Wp"], np.float32), np.asarray(inputs["bp"], np.float32)
    Wo, bo = np.asarray(inputs["Wo"], np.float32), np.asarray(inputs["bo"], np.float32)

    qT = np.ascontiguousarray(query.transpose(0, 2, 1))  # [B, IN, S]
    kT = np.ascontiguousarray(key.transpose(0, 2, 1))

    if "nc" not in _cache:
        _cache["nc"] = build()
    nc = _cache["nc"]

    in_maps = []
    for i in range(NCORES):
        hs = slice(i * HPC, (i + 1) * HPC)
        Wo_h = Wo.reshape(H, D, D)  # rows of Wo per head
        wh = np.einsum(
            "hde,hef->hdf",
            Wp[hs].astype(np.float64),
            Wo_h[hs].astype(np.float64),
        ).astype(np.float32)
        bias = (
            np.einsum("hd,hdf->f", bv[hs].astype(np.float64), wh.astype(np.float64))
            + np.einsum(
                "hd,hdf->f", bp[hs].astype(np.float64), Wo_h[hs].astype(np.float64)
            )
            + bo.astype(np.float64) / NCORES
        ).astype(np.float32)
        in_maps.append(
            {
                "qT": qT,
                "kT": kT,
                "wq": np.ascontiguousarray(Wq[hs]),
                "wk": np.ascontiguousarray(Wk[hs]),
                "wv": np.ascontiguousarray(Wv[hs]),
                "wh": wh,
                "bqT": np.ascontiguousarray(bq[hs].T),
                "bkT": np.ascontiguousarray(bk[hs].T),
                "biasv": bias.reshape(1, D),
                "oner": np.ones((1, QB), np.float32),
                "onem": np.ones((D, D), np.float32),
                "onemb": np.ones((D, D), ml_dtypes.bfloat16),
            }
        )

    res = run_bass_kernel_spmd(nc, in_maps, core_ids=list(range(NCORES)))
    _cache["last_result"] = res
    yT = np.concatenate([res.results[i]["out_y"] for i in range(NCORES)], axis=0)
    return np.ascontiguousarray(yT.T).reshape(B, S, D)


# revision 15
# speedup vs baseline: 1.3072x; 1.1046x over previous
"""Multi-head attention on 8 trn2 NeuronCores, head-parallel (2 heads/core).

Math per head h (reference semantics):
  Q = query @ Wq[h] + bq[h];  K = key @ Wk[h] + bk[h];  V = query @ Wv[h] + bv[h]
  P = exp(Q K^T / sqrt(D));  alpha = P / rowsum(P)
  ctx = alpha @ V;  y_h = (ctx @ Wp[h] + bp[h]) @ Wo[h]
  out = sum_h y_h + bo

Device-side formulation (all layouts transposed, f32 storage, f32r matmuls):
  Per core: project QT/KT/VT = W^T @ xT per head, attention with unnormalized
  softmax (rowsum via ones-matmul, normalization folded after PV), output
  y^T[e, tok] partial = sum_{h in core} W_h^T @ ctxn_h + bias ** ones, where
  W_h = Wp[h] @ Wo[h] (host-premultiplied) and bias collects bv/bp/bo terms.
  ReduceScatter across the 8 cores; host concatenates the shards.
"""

import sys

if "/opt/trn_rl_repo" not in sys.path:
    sys.path.insert(0, "/opt/trn_rl_repo")

import ml_dtypes
import numpy as np

import concourse.mybir as mybir
import concourse.tile as tile
from concourse import bacc
from concourse.bass_utils import run_bass_kernel_spmd

B, S = 4, 2048
IN, D, H = 1024, 128, 16
NCORES = 8
HPC = H // NCORES  # heads per core
NCH = IN // 128  # input chunks
TB = 512  # projection token block
NTB = S // TB
QB = 512  # attention query block
NQB = S // QB
KT = 128  # attention key tile
NKT = S // KT
ESH = D // NCORES  # output shard rows per core

f32 = mybir.dt.float32
f32r = mybir.dt.float32r
bf16 = mybir.dt.bfloat16
AF = mybir.ActivationFunctionType

_cache = {}


def build():
    nc = bacc.Bacc(None, target_bir_lowering=False, num_devices=NCORES)

    qT = nc.dram_tensor("qT", [B, IN, S], f32r, kind="ExternalInput")
    kT = nc.dram_tensor("kT", [B, IN, S], f32r, kind="ExternalInput")
    wq = nc.dram_tensor("wq", [HPC, IN, D], f32r, kind="ExternalInput")
    wk = nc.dram_tensor("wk", [HPC, IN, D], f32r, kind="ExternalInput")
    wv = nc.dram_tensor("wv", [HPC, IN, D], f32r, kind="ExternalInput")
    wh = nc.dram_tensor("wh", [HPC, D, D], f32r, kind="ExternalInput")
    bqT = nc.dram_tensor("bqT", [D, HPC], f32, kind="ExternalInput")
    bkT = nc.dram_tensor("bkT", [D, HPC], f32, kind="ExternalInput")
    biasv = nc.dram_tensor("biasv", [1, D], f32r, kind="ExternalInput")
    oner = nc.dram_tensor("oner", [1, QB], f32r, kind="ExternalInput")
    onemb = nc.dram_tensor("onemb", [D, D], bf16, kind="ExternalInput")

    out_y = nc.dram_tensor("out_y", [ESH, B * S], f32, kind="ExternalOutput")
    y_bounce = [nc.dram_tensor(f"y_bounce{b}", [D, S], f32) for b in range(B)]
    y_shard = [nc.dram_tensor(f"y_shard{b}", [ESH, S], f32) for b in range(B)]

    scale = 1.0 / float(np.sqrt(D))

    with tile.TileContext(nc) as tc:
        with (
            tc.tile_pool(name="const", bufs=1) as cpool,
            tc.tile_pool(name="xch", bufs=16) as xch,
            tc.tile_pool(name="qkv", bufs=2) as qkv,
            tc.tile_pool(name="work", bufs=2) as work,
            tc.tile_pool(name="pexpp", bufs=4) as pexpp,
            tc.tile_pool(name="ps", bufs=2, space="PSUM") as ps,
        ):
            # ---- resident constants ----
            wq_sb = cpool.tile([128, HPC, NCH, D], f32r, tag="wq_sb")
            wk_sb = cpool.tile([128, HPC, NCH, D], f32r, tag="wk_sb")
            wv_sb = cpool.tile([128, HPC, NCH, D], f32r, tag="wv_sb")
            for sb_t, dram_t in ((wq_sb, wq), (wk_sb, wk), (wv_sb, wv)):
                for h in range(HPC):
                    for c in range(NCH):
                        nc.sync.dma_start(
                            sb_t[:, h, c, :], dram_t[h, c * 128 : (c + 1) * 128, :]
                        )
            wh_sb = cpool.tile([128, HPC, D], f32r, tag="wh_sb")
            for h in range(HPC):
                nc.sync.dma_start(wh_sb[:, h, :], wh[h])
            bq_sb = cpool.tile([128, HPC], f32, tag="bq_sb")
            bk_sb = cpool.tile([128, HPC], f32, tag="bk_sb")
            nc.sync.dma_start(bq_sb[:], bqT[:])
            nc.sync.dma_start(bk_sb[:], bkT[:])
            biasv_sb = cpool.tile([1, D], f32r, tag="biasv_sb")
            oner_sb = cpool.tile([1, QB], f32r, tag="oner_sb")
            onemb_sb = cpool.tile([D, D], bf16, tag="onemb_sb")
            nc.sync.dma_start(onemb_sb[:], onemb[:])
            nc.sync.dma_start(biasv_sb[:], biasv[:])
            nc.sync.dma_start(oner_sb[:], oner[:])
            identb = cpool.tile([128, 128], bf16, tag="identb")
            from concourse.masks import make_identity

            make_identity(nc, identb[:])

            for b in range(B):
                # ---- projections: Q & V from qT, K from kT ----
                QT = [qkv.tile([128, S], f32r, tag=f"QT{h}", name=f"QT{h}") for h in range(HPC)]
                KTs = [qkv.tile([128, S], f32r, tag=f"KT{h}", name=f"KT{h}") for h in range(HPC)]
                Vn = [qkv.tile([128, S], bf16, tag=f"VN{h}", name=f"VN{h}") for h in range(HPC)]

                for tb in range(NTB):
                    sl = slice(tb * TB, (tb + 1) * TB)
                    chs = []
                    for c in range(NCH):
                        ch = xch.tile([128, TB], f32r, tag="xch")
                        nc.sync.dma_start(ch[:], qT[b, c * 128 : (c + 1) * 128, sl])
                        chs.append(ch)
                    pq = ps.tile([128, 2 * TB], f32, tag="pS", name="pq", bufs=2)
                    for h in range(HPC):
                        for c in range(NCH):
                            nc.tensor.matmul(
                                pq[:, h * TB : (h + 1) * TB],
                                wq_sb[:, h, c, :], chs[c][:],
                                start=(c == 0), stop=(c == NCH - 1),
                            )
                    for h in range(HPC):
                        with nc.allow_low_precision(reason="f32r PE operand"):
                            nc.vector.tensor_scalar_add(
                                QT[h][:, sl], pq[:, h * TB : (h + 1) * TB],
                                bq_sb[:, h : h + 1],
                            )
                    pv = ps.tile([128, 2 * TB], f32, tag="pS", name="pv", bufs=2)
                    for h in range(HPC):
                        for c in range(NCH):
                            nc.tensor.matmul(
                                pv[:, h * TB : (h + 1) * TB],
                                wv_sb[:, h, c, :], chs[c][:],
                                start=(c == 0), stop=(c == NCH - 1),
                            )
                    for h in range(HPC):
                        vstage = work.tile([128, TB], bf16, tag=f"vstage{h}")
                        with nc.allow_low_precision(reason="bf16 PV operand"):
                            nc.vector.tensor_copy(vstage[:], pv[:, h * TB : (h + 1) * TB])
                        for j in range(TB // 128):
                            ptr = ps.tile([128, 128], bf16, tag="pZ", name="ptr", bufs=2)
                            nc.tensor.transpose(
                                ptr[:], vstage[:, j * 128 : (j + 1) * 128], identb[:]
                            )
                            col = tb * TB + j * 128
                            with nc.allow_low_precision(reason="bf16 PV operand"):
                                nc.vector.tensor_copy(Vn[h][:, col : col + 128], ptr[:])

                for tb in range(NTB):
                    sl = slice(tb * TB, (tb + 1) * TB)
                    chs = []
                    for c in range(NCH):
                        ch = xch.tile([128, TB], f32r, tag="xch")
                        nc.sync.dma_start(ch[:], kT[b, c * 128 : (c + 1) * 128, sl])
                        chs.append(ch)
                    pk = ps.tile([128, 2 * TB], f32, tag="pS", name="pk", bufs=2)
                    for h in range(HPC):
                        for c in range(NCH):
                            nc.tensor.matmul(
                                pk[:, h * TB : (h + 1) * TB],
                                wk_sb[:, h, c, :], chs[c][:],
                                start=(c == 0), stop=(c == NCH - 1),
                            )
                    for h in range(HPC):
                        with nc.allow_low_precision(reason="f32r PE operand"):
                            nc.vector.tensor_scalar_add(
                                KTs[h][:, sl], pk[:, h * TB : (h + 1) * TB],
                                bk_sb[:, h : h + 1],
                            )

                # ---- attention: qblock pairs share 2-bank psum + one wide exp ----
                GPS_KT = set(range(0, 4))  # gpsimd takes early kt (tail-free)
                for qbp in range(NQB // 2):
                    q0 = qbp * 2 * QB
                    sl0 = slice(q0, q0 + QB)
                    sl1 = slice(q0 + QB, q0 + 2 * QB)
                    slp = slice(q0, q0 + 2 * QB)
                    pzs = [
                        ps.tile([128, QB], f32, tag="pZ", name="pz", bufs=2)
                        for _ in range(2)
                    ]
                    for h in range(HPC):
                        pctx0 = ps.tile([128, QB], f32, tag="pC", name="pctx0")
                        pctx1 = ps.tile([128, QB], f32, tag="pC", name="pctx1")
                        acc_d = work.tile([128, 2 * QB], bf16, tag="acc_d", name="acc_d")
                        acc_g = work.tile([128, 2 * QB], bf16, tag="acc_g", name="acc_g")
                        first_d, first_g = True, True
                        prev_d, prev_g = None, None
                        for kt in range(NKT):
                            ps2 = ps.tile([128, 2 * QB], f32, tag="pS", name="ps2", bufs=2)
                            ksl = slice(kt * 128, (kt + 1) * 128)
                            nc.tensor.matmul(
                                ps2[:, :QB], KTs[h][:, ksl], QT[h][:, sl0],
                                start=True, stop=True,
                            )
                            nc.tensor.matmul(
                                ps2[:, QB:], KTs[h][:, ksl], QT[h][:, sl1],
                                start=True, stop=True,
                            )
                            pexp = pexpp.tile([128, 2 * QB], bf16, tag="pexp", bufs=8)
                            nc.scalar.activation(pexp[:], ps2[:], AF.Exp, scale=scale)
                            nc.tensor.matmul(
                                pctx0[:], Vn[h][:, ksl], pexp[:, :QB],
                                start=(kt == 0), stop=(kt == NKT - 1),
                            )
                            nc.tensor.matmul(
                                pctx1[:], Vn[h][:, ksl], pexp[:, QB:],
                                start=(kt == 0), stop=(kt == NKT - 1),
                            )
                            with nc.allow_low_precision(reason="f32r PE operand"):
                                if kt in GPS_KT:
                                    if first_g and prev_g is None:
                                        prev_g = pexp
                                    elif first_g:
                                        nc.gpsimd.tensor_add(
                                            acc_g[:], prev_g[:], pexp[:]
                                        )
                                        first_g = False
                                    else:
                                        nc.gpsimd.tensor_add(
                                            acc_g[:], acc_g[:], pexp[:]
                                        )
                                else:
                                    if first_d and prev_d is None:
                                        prev_d = pexp
                                    elif first_d:
                                        nc.vector.tensor_add(
                                            acc_d[:], prev_d[:], pexp[:]
                                        )
                                        first_d = False
                                    else:
                                        nc.vector.tensor_add(
                                            acc_d[:], acc_d[:], pexp[:]
                                        )
                        # rowsum+broadcast via all-ones matmul, both halves
                        pbc = ps.tile([128, 2 * QB], f32, tag="pS", name="pbc", bufs=2)
                        for half, hsl in ((0, slice(0, QB)), (1, slice(QB, 2 * QB))):
                            nc.tensor.matmul(
                                pbc[:, hsl], onemb_sb[:], acc_d[:, hsl],
                                start=True, stop=False,
                            )
                            nc.tensor.matmul(
                                pbc[:, hsl], onemb_sb[:], acc_g[:, hsl],
                                start=False, stop=True,
                            )
                        rsbr = work.tile([128, 2 * QB], f32, tag="rsbr", name="rsbr", bufs=1)
                        nc.vector.reciprocal_approx_fast(out=rsbr[:], in_=pbc[:])
                        ctxn = work.tile([128, 2 * QB], f32r, tag="ctxn", name="ctxn")
                        with nc.allow_low_precision(reason="f32r PE operand"):
                            nc.vector.tensor_mul(ctxn[:, :QB], pctx0[:], rsbr[:, :QB])
                            nc.vector.tensor_mul(ctxn[:, QB:], pctx1[:], rsbr[:, QB:])
                        nc.tensor.matmul(
                            pzs[0][:], wh_sb[:, h, :], ctxn[:, :QB],
                            start=(h == 0), stop=False,
                        )
                        nc.tensor.matmul(
                            pzs[1][:], wh_sb[:, h, :], ctxn[:, QB:],
                            start=(h == 0), stop=False,
                        )
                    for half in range(2):
                        nc.tensor.matmul(
                            pzs[half][:], biasv_sb[:], oner_sb[:], start=False, stop=True
                        )
                        ytile = work.tile([128, QB], f32, tag="ytile")
                        nc.vector.tensor_copy(ytile[:], pzs[half][:])
                        col = q0 + half * QB
                        nc.sync.dma_start(y_bounce[b][:, col : col + QB], ytile[:])

                nc.gpsimd.collective_compute(
                    "ReduceScatter",
                    mybir.AluOpType.add,
                    replica_groups=[list(range(NCORES))],
                    ins=[y_bounce[b][:].opt()],
                    outs=[y_shard[b][:].opt()],
                )
                nc.sync.dma_start(out_y[:, b * S : (b + 1) * S], y_shard[b][:])


    nc.compile()
    return nc


def kernel(**inputs):
    query = np.asarray(inputs["query"], np.float32)
    key = np.asarray(inputs["key"], np.float32)
    Wq, bq = np.asarray(inputs["Wq"], np.float32), np.asarray(inputs["bq"], np.float32)
    Wk, bk = np.asarray(inputs["Wk"], np.float32), np.asarray(inputs["bk"], np.float32)
    Wv, bv = np.asarray(inputs["Wv"], np.float32), np.asarray(inputs["bv"], np.float32)
    Wp, bp = np.asarray(inputs["Wp"], np.float32), np.asarray(inputs["bp"], np.float32)
    Wo, bo = np.asarray(inputs["Wo"], np.float32), np.asarray(inputs["bo"], np.float32)

    qT = np.ascontiguousarray(query.transpose(0, 2, 1))  # [B, IN, S]
    kT = np.ascontiguousarray(key.transpose(0, 2, 1))

    if "nc" not in _cache:
        _cache["nc"] = build()
    nc = _cache["nc"]

    in_maps = []
    for i in range(NCORES):
        hs = slice(i * HPC, (i + 1) * HPC)
        Wo_h = Wo.reshape(H, D, D)  # rows of Wo per head
        wh = np.einsum(
            "hde,hef->hdf",
            Wp[hs].astype(np.float64),
            Wo_h[hs].astype(np.float64),
        ).astype(np.float32)
        bias = (
            np.einsum("hd,hdf->f", bv[hs].astype(np.float64), wh.astype(np.float64))
            + np.einsum(
                "hd,hdf->f", bp[hs].astype(np.float64), Wo_h[hs].astype(np.float64)
            )
            + bo.astype(np.float64) / NCORES
        ).astype(np.float32)
        in_maps.append(
            {
                "qT": qT,
                "kT": kT,
                "wq": np.ascontiguousarray(Wq[hs]),
                "wk": np.ascontiguousarray(Wk[hs]),
                "wv": np.ascontiguousarray(Wv[hs]),
                "wh": wh,
                "bqT": np.ascontiguousarray(bq[hs].T),
                "bkT": np.ascontiguousarray(bk[hs].T),
                "biasv": bias.reshape(1, D),
                "oner": np.ones((1, QB), np.float32),
                "onemb": np.ones((D, D), ml_dtypes.bfloat16),
            }
        )

    res = run_bass_kernel_spmd(nc, in_maps, core_ids=list(range(NCORES)))
    _cache["last_result"] = res
    yT = np.concatenate([res.results[i]["out_y"] for i in range(NCORES)], axis=0)
    return np.ascontiguousarray(yT.T).reshape(B, S, D)
